# revision 1
# baseline (speedup 1.0000x reference)
"""Distributed MultiHeadAttention kernel for 8 TRN2 NeuronCores.

Sharding: core c -> batch b=c//4, head-group g=c%4 (heads 4g..4g+3).
Each core:
  - projects q/k/v for its 4 heads (fp32r matmuls, transposed layouts),
  - computes attention with transposed scores [k,q] so softmax needs no
    attention-matrix transpose (no max subtraction: scores are bounded;
    mask applied multiplicatively in bf16 after exp; softmax denominator
    via a ones-column appended to V),
  - AllToAll exchanges normalized per-head context so each core ends up
    with the full 1024-dim context for its 512-token output slice,
  - O-projection + residual + LayerNorm on that slice.
Host concatenates the 8 output shards.
"""

import sys

for p in ("/opt/trn_rl_repo",):
    if p not in sys.path:
        sys.path.append(p)

import numpy as np
import ml_dtypes

B, S, D, H = 2, 2048, 1024, 16
DK = 64          # head dim
HPC = 4          # heads per core
G = 4            # cores per batch group
TOK = S // G     # 512 output tokens per core
LN_EPS = 1e-5
NCORES = 8

_CACHE = {}


def _build_nc(sim=False):
    import concourse.mybir as mybir
    import concourse.tile as tile
    from concourse import bacc

    f32 = mybir.dt.float32
    f32r = mybir.dt.float32r
    bf16 = mybir.dt.bfloat16
    Exp = mybir.ActivationFunctionType.Exp
    Sqrt = mybir.ActivationFunctionType.Sqrt

    nc = bacc.Bacc("TRN2", target_bir_lowering=False, debug=False, num_devices=1 if sim else NCORES)

    qt = nc.dram_tensor("qt", [D, S], f32r, kind="ExternalInput").ap()        # Q[b].T
    qres = nc.dram_tensor("qres", [TOK, D], f32, kind="ExternalInput").ap()  # Q slice + bo
    maskt = nc.dram_tensor("maskt", [S, S], bf16, kind="ExternalInput").ap() # keep-mask.T
    wq = nc.dram_tensor("wq", [D, HPC * DK], f32r, kind="ExternalInput").ap() # pre-scaled 1/8
    wk = nc.dram_tensor("wk", [D, HPC * DK], f32r, kind="ExternalInput").ap()
    wv = nc.dram_tensor("wv", [D, HPC * DK], f32r, kind="ExternalInput").ap()
    bqk = nc.dram_tensor("bqk", [2 * HPC * DK], f32, kind="ExternalInput").ap()
    bv = nc.dram_tensor("bv", [HPC * DK], f32r, kind="ExternalInput").ap()
    wo = nc.dram_tensor("wo", [2 * D, D], f32r, kind="ExternalInput").ap()  # zero-padded per batch
    gamma = nc.dram_tensor("gamma", [D], f32, kind="ExternalInput").ap()
    beta = nc.dram_tensor("beta", [D], f32, kind="ExternalInput").ap()
    out = nc.dram_tensor("out", [TOK, D], f32, kind="ExternalOutput").ap()

    RG = [[0, 1, 2, 3], [4, 5, 6, 7]]
    NKC = D // 128    # 8 contraction chunks for d_model
    NTC = S // 128    # 16 token chunks
    NQT = S // 512    # 4 query tiles
    VW = HPC * (DK + 1)  # 260: v + per-head ones column

    with tile.TileContext(nc) as tc:
        with (
            tc.tile_pool(name="dram", bufs=1, space="DRAM") as dpool,
            tc.tile_pool(name="consts", bufs=1) as cpool,
            tc.tile_pool(name="qkv", bufs=1) as qkvpool,
        ):
            ctx_local = dpool.tile([2 * G * HPC * DK, TOK], f32r)
            ctx_glob = dpool.tile([2 * G * HPC * DK, TOK], f32r)

            # ---- constants ----
            ones1_f = cpool.tile([1, 128], f32)
            nc.vector.memset(ones1_f[:], 1.0)
            ones1 = cpool.tile([1, 128], f32r)
            nc.vector.tensor_copy(out=ones1[:], in_=ones1_f[:])
            eps_t = cpool.tile([128, 1], f32)
            nc.vector.memset(eps_t[:], LN_EPS)
            bqk_sb = cpool.tile([128, 2, 2], f32)  # [p, proj(q/k), hp]
            nc.sync.dma_start(bqk_sb[:], bqk.rearrange("(w hp p) -> p w hp", p=128, hp=2))
            bv_sb = cpool.tile([1, HPC * DK], f32r)
            nc.sync.dma_start(bv_sb[:], bv.unsqueeze(0))
            grow = cpool.tile([1, D], f32)
            nc.sync.dma_start(grow[:], gamma.unsqueeze(0))
            brow = cpool.tile([1, D], f32)
            nc.sync.dma_start(brow[:], beta.unsqueeze(0))
            gamma_bc = cpool.tile([128, D], f32)
            nc.gpsimd.partition_broadcast(gamma_bc[:], grow[:])
            beta_bc = cpool.tile([128, D], f32)
            nc.gpsimd.partition_broadcast(beta_bc[:], brow[:])

            # ---- persistent qkv activations ----
            qk_sb = qkvpool.tile([128, 2, 2, S], f32r)   # [p, proj(q/k), hp, tokens]
            v_sb = qkvpool.tile([128, NTC, VW], bf16)   # [p, token-chunk, 4*(64+1)]
            v4 = v_sb.rearrange("p t (h x) -> p t h x", x=DK + 1)
            nc.vector.memset(v4[:, :, :, DK : DK + 1], 1.0)

            # ================= Phase 1: projections =================
            with (
                tc.tile_pool(name="qtp", bufs=1) as qtp,
                tc.tile_pool(name="wp", bufs=1) as wp,
                tc.tile_pool(name="pps", bufs=2, space="PSUM") as pps,
                tc.tile_pool(name="ppv", bufs=2, space="PSUM") as ppv,
            ):
                qt_sb = qtp.tile([128, NKC, S], f32r)
                for kc in range(NKC):
                    nc.sync.dma_start(
                        qt_sb[:, kc, :],
                        qt.rearrange("(kc p) s -> p kc s", p=128)[:, kc, :],
                    )
                wq_sb = wp.tile([128, NKC, HPC * DK], f32r)
                wk_sb = wp.tile([128, NKC, HPC * DK], f32r)
                wv_sb = wp.tile([128, NKC, HPC * DK], f32r)
                for w_ap, w_t in ((wq, wq_sb), (wk, wk_sb), (wv, wv_sb)):
                    nc.sync.dma_start(w_t[:], w_ap.rearrange("(kc p) c -> p kc c", p=128))

                # q/k projections -> transposed [dims, tokens] layout
                for proj, w_t in ((0, wq_sb), (1, wk_sb)):
                    for hp in range(2):
                        for nt in range(NQT):
                            ps = pps.tile([128, 512], f32, name="ps_qk", tag="ps_qk")
                            for kc in range(NKC):
                                nc.tensor.matmul(
                                    ps[:],
                                    w_t[:, kc, hp * 128 : (hp + 1) * 128],
                                    qt_sb[:, kc, nt * 512 : (nt + 1) * 512],
                                    start=(kc == 0),
                                    stop=(kc == NKC - 1),
                                )
                            nc.vector.tensor_scalar_add(
                                out=qk_sb[:, proj, hp, nt * 512 : (nt + 1) * 512],
                                in0=ps[:],
                                scalar1=bqk_sb[:, proj, hp : hp + 1],
                            )

                # v projection -> natural [tokens, dims] layout (bf16, ones col)
                for tcn in range(NTC):
                    psv = ppv.tile([128, HPC * DK], f32, name="psv", tag="psv")
                    for kc in range(NKC):
                        nc.tensor.matmul(
                            psv[:],
                            qt_sb[:, kc, tcn * 128 : (tcn + 1) * 128],
                            wv_sb[:, kc, :],
                            start=(kc == 0),
                            stop=False,
                        )
                    nc.tensor.matmul(
                        psv[:],
                        ones1[:, :],
                        bv_sb[:, :],
                        start=False,
                        stop=True,
                    )
                    nc.vector.tensor_copy(
                        out=v4[:, tcn, :, 0:DK],
                        in_=psv.rearrange("p (h x) -> p h x", x=DK),
                    )

            # ================= Phase 2: attention =================
            with (
                tc.tile_pool(name="maskp", bufs=2) as maskp,
                tc.tile_pool(name="ep", bufs=8) as ep,
                tc.tile_pool(name="ap_", bufs=8) as ap_,
                tc.tile_pool(name="sps", bufs=4, space="PSUM") as spsp,
                tc.tile_pool(name="cps", bufs=2, space="PSUM") as cpsp,
                tc.tile_pool(name="nrm", bufs=4) as nrm,
            ):
                for qt_i in range(NQT):
                    mq = maskp.tile([128, NTC, 512], bf16, name="mq", tag="mq")
                    nc.sync.dma_start(
                        mq[:],
                        maskt[:, qt_i * 512 : (qt_i + 1) * 512].rearrange(
                            "(kc p) q -> p kc q", p=128
                        ),
                    )
                    for hp in range(2):
                        cps = [
                            cpsp.tile([DK + 1, 512], f32, name=f"cps{h2}", tag=f"cps{h2}")
                            for h2 in range(2)
                        ]
                        for kc in range(NTC):
                            for h2 in range(2):
                                sp = spsp.tile([128, 512], f32, name="sp", tag="sp")
                                nc.tensor.matmul(
                                    sp[:],
                                    qk_sb[
                                        64 * h2 : 64 * (h2 + 1),
                                        1,
                                        hp,
                                        kc * 128 : (kc + 1) * 128,
                                    ],
                                    qk_sb[
                                        64 * h2 : 64 * (h2 + 1),
                                        0,
                                        hp,
                                        qt_i * 512 : (qt_i + 1) * 512,
                                    ],
                                    start=True,
                                    stop=True,
                                )
                                e = ep.tile([128, 512], bf16, name="e", tag="e")
                                nc.scalar.activation(e[:], sp[:], Exp)
                                a = ap_.tile([128, 512], bf16, name="a", tag="a")
                                nc.vector.tensor_mul(a[:], e[:], mq[:, kc, :])
                                h = 2 * hp + h2
                                nc.tensor.matmul(
                                    cps[h2][:],
                                    v_sb[:, kc, h * (DK + 1) : (h + 1) * (DK + 1)],
                                    a[:],
                                    start=(kc == 0),
                                    stop=(kc == NTC - 1),
                                )
                        for h2 in range(2):
                            h = 2 * hp + h2
                            srec = nrm.tile([1, 512], f32, name="srec", tag="srec")
                            nc.vector.reciprocal(srec[:], cps[h2][DK : DK + 1, :])
                            rbc = nrm.tile([DK, 512], f32, name="rbc", tag="rbc")
                            nc.gpsimd.partition_broadcast(rbc[:], srec[:])
                            ctxn = nrm.tile([DK, 512], f32r, name="ctxn", tag="ctxn")
                            nc.vector.tensor_mul(ctxn[:], cps[h2][0:DK, :], rbc[:])
                            for half in range(2):
                                base = half * (G * HPC * DK) + qt_i * (HPC * DK) + h * DK
                                nc.sync.dma_start(
                                    ctx_local[base : base + DK, :], ctxn[:]
                                )

            # ================= Phase 3: AllToAll =================
            if sim:
                nc.sync.dma_start(ctx_glob[:], ctx_local[:])
            else:
                nc.gpsimd.collective_compute(
                    "AllToAll",
                    mybir.AluOpType.bypass,
                    replica_groups=[list(range(NCORES))],
                    ins=[ctx_local.opt()],
                    outs=[ctx_glob.opt()],
                )

            # ================= Phase 4: O-proj + residual + LN =================
            with (
                tc.tile_pool(name="ctxp", bufs=1) as ctxp,
                tc.tile_pool(name="wop", bufs=1) as wop,
                tc.tile_pool(name="qrp", bufs=1) as qrp,
                tc.tile_pool(name="ops", bufs=4, space="PSUM") as opsp,
                tc.tile_pool(name="oln", bufs=3) as oln,
            ):
                NOC = 2 * D // 128  # 16 contraction chunks (half are zero-Wo)
                ctx_sb = ctxp.tile([128, NOC, TOK], f32r)
                nc.sync.dma_start(
                    ctx_sb[:], ctx_glob.rearrange("(kc p) t -> p kc t", p=128)
                )
                wo_sb = wop.tile([128, NOC, D], f32r)
                nc.sync.dma_start(wo_sb[:], wo.rearrange("(kc p) d -> p kc d", p=128))
                qres_sb = qrp.tile([128, TOK // 128, D], f32)
                nc.sync.dma_start(
                    qres_sb[:], qres.rearrange("(mt p) d -> p mt d", p=128)
                )

                for mt in range(TOK // 128):
                    osb = oln.tile([128, D], f32, name="osb", tag="osb")
                    for nt in range(2):
                        pso = opsp.tile([128, 512], f32, name="pso", tag="pso")
                        for kc in range(NOC):
                            nc.tensor.matmul(
                                pso[:],
                                ctx_sb[:, kc, mt * 128 : (mt + 1) * 128],
                                wo_sb[:, kc, nt * 512 : (nt + 1) * 512],
                                start=(kc == 0),
                                stop=(kc == NOC - 1),
                            )
                        nc.vector.tensor_add(
                            out=osb[:, nt * 512 : (nt + 1) * 512],
                            in0=pso[:],
                            in1=qres_sb[:, mt, nt * 512 : (nt + 1) * 512],
                        )
                    stats = oln.tile([128, 2, 6], f32, name="stats", tag="stats")
                    for sg in range(2):
                        nc.vector.bn_stats(
                            out=stats[:, sg, :], in_=osb[:, sg * 512 : (sg + 1) * 512]
                        )
                    mv = oln.tile([128, 2], f32, name="mv", tag="mv")
                    nc.vector.bn_aggr(out=mv[:], in_=stats[:])
                    rstd = oln.tile([128, 1], f32, name="rstd", tag="rstd")
                    nc.scalar.activation(rstd[:], mv[:, 1:2], Sqrt, bias=eps_t[:])
                    nc.vector.reciprocal(rstd[:], rstd[:])
                    y = oln.tile([128, D], f32, name="y", tag="y")
                    nc.vector.tensor_scalar(
                        out=y[:],
                        in0=osb[:],
                        scalar1=mv[:, 0:1],
                        scalar2=rstd[:],
                        op0=mybir.AluOpType.subtract,
                        op1=mybir.AluOpType.mult,
                    )
                    nc.vector.tensor_mul(y[:], y[:], gamma_bc[:])
                    nc.vector.tensor_add(y[:], y[:], beta_bc[:])
                    nc.sync.dma_start(out[mt * 128 : (mt + 1) * 128, :], y[:])

    nc.compile()
    return nc


def _get_nc():
    if "nc" not in _CACHE:
        _CACHE["nc"] = _build_nc()
    return _CACHE["nc"]


def make_in_maps(inputs):
    Q = np.asarray(inputs["Q"], np.float32)
    mask = np.asarray(inputs["attn_mask"])
    Wq = np.asarray(inputs["Wq"], np.float32)
    Wk = np.asarray(inputs["Wk"], np.float32)
    Wv = np.asarray(inputs["Wv"], np.float32)
    Wo = np.asarray(inputs["Wo"], np.float32)
    bq = np.asarray(inputs["bq"], np.float32)
    bk = np.asarray(inputs["bk"], np.float32)
    bv = np.asarray(inputs["bv"], np.float32)
    bo = np.asarray(inputs["bo"], np.float32)
    gamma = np.asarray(inputs["gamma"], np.float32)
    beta = np.asarray(inputs["beta"], np.float32)
    scale = np.float32(1.0 / np.sqrt(DK))

    in_maps = []
    for c in range(NCORES):
        b, g = c // G, c % G
        hs = slice(g * HPC * DK, (g + 1) * HPC * DK)
        qtb = np.ascontiguousarray(Q[b].T)
        mtb = np.ascontiguousarray((~mask[b]).T).astype(ml_dtypes.bfloat16)
        wo_eff = np.zeros((2 * D, D), np.float32)
        wo_eff[b * D : (b + 1) * D] = Wo
        in_maps.append(
            {
                "qt": qtb,
                "qres": np.ascontiguousarray(Q[b, g * TOK : (g + 1) * TOK]) + bo,
                "maskt": mtb,
                "wq": np.ascontiguousarray(Wq[:, hs]) * scale,
                "wk": np.ascontiguousarray(Wk[:, hs]),
                "wv": np.ascontiguousarray(Wv[:, hs]),
                "bqk": np.concatenate([bq[hs] * scale, bk[hs]]),
                "bv": np.ascontiguousarray(bv[hs]),
                "wo": wo_eff,
                "gamma": gamma,
                "beta": beta,
            }
        )
    return in_maps


def kernel(**inputs):
    from concourse.bass_utils import run_bass_kernel_spmd

    nc = _get_nc()
    in_maps = make_in_maps(inputs)
    res = run_bass_kernel_spmd(nc, in_maps, core_ids=list(range(NCORES)))
    out = np.empty((B, S, D), np.float32)
    for c in range(NCORES):
        b, g = c // G, c % G
        out[b, g * TOK : (g + 1) * TOK] = res.results[c]["out"]
    return out



# revision 49
# speedup vs baseline: 1.8338x; 1.8338x over previous
"""Distributed MultiHeadAttention kernel for 8 TRN2 NeuronCores.

Sharding: core c -> batch b=c//4, query-token slice g=c%4 (512 tokens).
No collectives: each core redundantly computes k/v for its whole batch
(cheaper than an AllGather under the runtime model) and produces its own
512-token output slice locally.

Per core:
  - q/k/v projections in fp8e4 with DoubleRow matmuls (weights host-scaled
    by 32 into fp8's normal range; descaled at PSUM evacuation),
  - scores in bf16, transposed [k, q] layout so softmax needs no
    attention-matrix transpose (no max subtraction: scores are bounded),
  - exp on ScalarE over 2-bank PSUM groups; keep-mask applied
    multiplicatively in bf16 on VectorE,
  - context accumulated in natural [q, d] layout with a ones-column per
    head giving the softmax denominator for free; normalization is a
    per-partition tensor_scalar at evacuation,
  - context transposed via the DMA xbar (bf16), converted to fp8,
  - O-projection in fp8 DoubleRow + residual (host-prescaled by 32; the
    final LayerNorm is scale-invariant) + LayerNorm.
Host concatenates the 8 output shards.
"""

import sys

for p in ("/opt/trn_rl_repo",):
    if p not in sys.path:
        sys.path.append(p)

import numpy as np
import ml_dtypes

B, S, D, H = 2, 2048, 1024, 16
DK = 64          # head dim
G = 4            # query-shards per batch
TOK = S // G     # 512 output tokens per core
LN_EPS = 1e-5
NCORES = 8
SW = 32.0        # fp8 weight scale
NKC = D // 128   # 8 contraction chunks over d_model
NTC = S // 128   # 16 key-token chunks
HPAIR = 8        # head pairs (2 heads / 128 partitions)
VW = DK + 1      # 65: v + ones column per head

_CACHE = {}

# engine-balance tunables
SCHRAUD_KCS = (3, 9, 14)   # kcs whose exp runs on DVE (Schraudolph)
POOL_MASK_COLS = 128       # trailing mask columns handled by GpSimd
EVAC_MOD = 2               # 1 of EVAC_MOD proj evacuations goes to ScalarE
CTXNORM_ACT = 1            # h2 values whose ctx-norm runs on ScalarE (bitmask)


def _build_nc(sim=False, use_bias=False, use_gb=False):
    import concourse.mybir as mybir
    import concourse.tile as tile
    from concourse import bacc

    f32 = mybir.dt.float32
    bf16 = mybir.dt.bfloat16
    f8 = mybir.dt.float8e4
    Exp = mybir.ActivationFunctionType.Exp
    Sqrt = mybir.ActivationFunctionType.Sqrt
    Square = mybir.ActivationFunctionType.Square
    Copy = mybir.ActivationFunctionType.Copy
    i16 = mybir.dt.int16
    DR = mybir.MatmulPerfMode.DoubleRow
    MUL = mybir.AluOpType.mult
    ADD = mybir.AluOpType.add
    SUB = mybir.AluOpType.subtract

    nc = bacc.Bacc("TRN2", target_bir_lowering=False, debug=False,
                   num_devices=1 if sim else NCORES)

    qt = nc.dram_tensor("qt", [D, S], f8, kind="ExternalInput").ap()       # Q[b].T fp8
    qtl = nc.dram_tensor("qtl", [D, TOK], f8, kind="ExternalInput").ap()   # local slice.T
    maskt = nc.dram_tensor("maskt", [S, TOK], bf16, kind="ExternalInput").ap()
    wq = nc.dram_tensor("wq", [D, D], f8, kind="ExternalInput").ap()       # *SW/ (scores /8 at evac)
    wk = nc.dram_tensor("wk", [D, D], f8, kind="ExternalInput").ap()
    wv = nc.dram_tensor("wv", [D, D], f8, kind="ExternalInput").ap()
    wo = nc.dram_tensor("wo", [D, D], f8, kind="ExternalInput").ap()
    qres = nc.dram_tensor("qres", [TOK, D], f32, kind="ExternalInput").ap()  # (Q+bo)*SW
    wors = nc.dram_tensor("wors", [D, 1], f8, kind="ExternalInput").ap()     # row-sums of wo8
    qrsm = nc.dram_tensor("qrsm", [TOK], f32, kind="ExternalInput").ap()     # sum(qres)/D
    ident = nc.dram_tensor("ident", [128, 128], bf16, kind="ExternalInput").ap()
    out = nc.dram_tensor("out", [TOK, D], f32, kind="ExternalOutput").ap()
    if use_bias:
        bq = nc.dram_tensor("bq", [D], f32, kind="ExternalInput").ap()   # /8
        bk = nc.dram_tensor("bk", [D], f32, kind="ExternalInput").ap()
        bv = nc.dram_tensor("bv", [1, D], bf16, kind="ExternalInput").ap()  # raw
    if use_gb:
        gamma = nc.dram_tensor("gamma", [D], f32, kind="ExternalInput").ap()
        beta = nc.dram_tensor("beta", [D], f32, kind="ExternalInput").ap()

    with tile.TileContext(nc) as tc:
        with (
            tc.tile_pool(name="consts", bufs=1) as cpool,
            tc.tile_pool(name="persist", bufs=1) as ppool,
        ):
            # ---- constants ----
            eps_t = cpool.tile([128, 1], f32)
            nc.vector.memset(eps_t[:], LN_EPS * SW * SW)
            if use_bias:
                ones1 = cpool.tile([1, 128], bf16)
                nc.vector.memset(ones1[:], 1.0)
                bq_sb = cpool.tile([128, NKC], f32)
                nc.sync.dma_start(bq_sb[:], bq.rearrange("(c p) -> p c", p=128))
                bk_sb = cpool.tile([128, NKC], f32)
                nc.sync.dma_start(bk_sb[:], bk.rearrange("(c p) -> p c", p=128))
                bv_sb = cpool.tile([1, D], bf16)
                nc.sync.dma_start(bv_sb[:], bv)
            if use_gb:
                grow = cpool.tile([1, D], f32)
                nc.sync.dma_start(grow[:], gamma.unsqueeze(0))
                brow = cpool.tile([1, D], f32)
                nc.sync.dma_start(brow[:], beta.unsqueeze(0))
                gamma_bc = cpool.tile([128, D], f32)
                nc.gpsimd.partition_broadcast(gamma_bc[:], grow[:])
                beta_bc = cpool.tile([128, D], f32)
                nc.gpsimd.partition_broadcast(beta_bc[:], brow[:])

            # ---- persistent activations (live into the tail phase) ----
            wo_sb = ppool.tile([128, NKC, D], f8)
            wors_sb = ppool.tile([128, NKC, 1], f8)
            qrsm_sb = ppool.tile([128, TOK // 128], f32)
            qres_sb = ppool.tile([128, TOK // 128, D], f32)
            ctxn = ppool.tile([128, TOK // 128, D], bf16)   # natural [q, d]
            ctxt = ppool.tile([128, NKC, TOK], bf16)        # transposed [d, q]
            ctxt8 = ppool.tile([128, NKC, TOK], f8)
            ident_sb = ppool.tile([128, 128], bf16)

            # ========== Phases 1+2: projections (fp8 DR) + attention ======
            with (
                tc.tile_pool(name="attn", bufs=1) as apool,
                tc.tile_pool(name="sps", bufs=3, space="PSUM") as spsp,
                tc.tile_pool(name="cps", bufs=1, space="PSUM") as cpsp,
                tc.tile_pool(name="ep", bufs=4) as ep,
                tc.tile_pool(name="ap_", bufs=4) as ap_,
                tc.tile_pool(name="nrm", bufs=2) as nrm,
            ):
                # PE p-state warmup: dummy matmuls on a const tile keep
                # the tensor engine busy from t=0 so the projections run at
                # full clock once their DMAs land
                warm = apool.tile([1, 512], bf16)
                nc.vector.memset(warm[:], 0.0)
                wps = spsp.tile([128, 2, 512], f32, name="sp", tag="sp")
                for _ in range(10):
                    nc.tensor.matmul(wps[0:1, 0, :], warm[:, 0:1], warm[:],
                                     start=True, stop=True)

                qt_sb = apool.tile([128, NKC, S], f8)       # Q.T, all tokens
                qtl_sb = apool.tile([128, NKC, TOK], f8)    # Q.T, local queries
                wk_sb = apool.tile([128, NKC, D], f8)
                wv_sb = apool.tile([128, NKC, D], f8)
                wq_sb = apool.tile([128, NKC, D], f8)
                k_sb = apool.tile([128, HPAIR, S], bf16)    # [dims(2h,dk), tokens]
                v_sb = apool.tile([128, NTC, H, VW], bf16)  # [tok128, chunk, head, v|1]
                q_sb = apool.tile([128, HPAIR, TOK], bf16)
                mask_sb = apool.tile([128, NTC, TOK], bf16)

                # load order matters: q-path first so attention starts early
                wqr = wq.rearrange("(c p) d -> p c d", p=128)
                wkr = wk.rearrange("(c p) d -> p c d", p=128)
                qrr = qt.rearrange("(c p) s -> p c s", p=128)
                nc.sync.dma_start(qtl_sb[:], qtl.rearrange("(c p) s -> p c s", p=128))
                nc.sync.dma_start(wq_sb[:, :, 0:128], wqr[:, :, 0:128])
                nc.sync.dma_start(qt_sb[:, :, 0:512], qrr[:, :, 0:512])
                nc.sync.dma_start(wk_sb[:, :, 0:128], wkr[:, :, 0:128])
                nc.sync.dma_start(wq_sb[:, :, 128:D], wqr[:, :, 128:D])
                nc.sync.dma_start(qt_sb[:, :, 512:S], qrr[:, :, 512:S])
                nc.sync.dma_start(wk_sb[:, :, 128:D], wkr[:, :, 128:D])
                mrr = maskt.rearrange("(c p) q -> p c q", p=128)
                nc.sync.dma_start(mask_sb[:, 0:4, :], mrr[:, 0:4, :])
                nc.sync.dma_start(wv_sb[:], wv.rearrange("(c p) d -> p c d", p=128))
                nc.sync.dma_start(mask_sb[:, 4:NTC, :], mrr[:, 4:NTC, :])
                nc.sync.dma_start(qres_sb[:], qres.rearrange("(t p) d -> p t d", p=128))
                nc.sync.dma_start(wo_sb[:], wo.rearrange("(c p) d -> p c d", p=128))
                nc.sync.dma_start(wors_sb[:], wors.rearrange("(c p) x -> p c x", p=128))
                nc.sync.dma_start(qrsm_sb[:], qrsm.rearrange("(t p) -> p t", p=128))
                nc.sync.dma_start(ident_sb[:], ident)
                nc.vector.memset(v_sb[:, :, :, DK:VW], 1.0)
                evac_flip = [0]

                def evac(dst, src, scale, bias_ap):
                    if use_bias and bias_ap is not None:
                        nc.vector.tensor_scalar(
                            out=dst, in0=src, scalar1=scale,
                            scalar2=bias_ap, op0=MUL, op1=ADD)
                        return
                    evac_flip[0] = (evac_flip[0] + 1) % EVAC_MOD
                    if evac_flip[0] != 0:
                        nc.vector.tensor_scalar_mul(dst, src, scale)
                    else:
                        nc.scalar.mul(dst, src, scale)

                def kproj(hs):
                    for tcn in range(4):
                        ps = spsp.tile([128, 2, 512], f32, name="sp", tag="sp")[:, 0, :]
                        for j in range(NKC // 2):
                            nc.tensor.matmul(
                                ps[:],
                                wk_sb[:, 2 * j : 2 * j + 2, hs * 128 : (hs + 1) * 128],
                                qt_sb[:, 2 * j : 2 * j + 2, tcn * 512 : (tcn + 1) * 512],
                                start=(j == 0), stop=(j == NKC // 2 - 1),
                                perf_mode=DR,
                            )
                        evac(k_sb[:, hs, tcn * 512 : (tcn + 1) * 512], ps[:],
                             1.0 / SW, bk_sb[:, hs : hs + 1] if use_bias else None)

                def vproj(tcn):
                    for half in range(2):
                        ps = spsp.tile([128, 2, 512], f32, name="sp", tag="sp")[:, 0, :]
                        last = NKC // 2 - 1
                        for j in range(NKC // 2):
                            nc.tensor.matmul(
                                ps[:],
                                qt_sb[:, 2 * j : 2 * j + 2, tcn * 128 : (tcn + 1) * 128],
                                wv_sb[:, 2 * j : 2 * j + 2, half * 512 : (half + 1) * 512],
                                start=(j == 0),
                                stop=(not use_bias and j == last),
                                perf_mode=DR,
                            )
                        if use_bias:
                            nc.tensor.matmul(
                                ps[:], ones1[:, :],
                                bv_sb[:, half * 512 : (half + 1) * 512],
                                start=False, stop=True)
                        evac(v_sb[:, tcn, half * 8 : half * 8 + 8, 0:DK],
                             ps.rearrange("p (h x) -> p h x", x=DK), 1.0 / SW, None)

                def qproj(hp):
                    ps = spsp.tile([128, 2, 512], f32, name="sp", tag="sp")[:, 0, :]
                    for j in range(NKC // 2):
                        nc.tensor.matmul(
                            ps[:],
                            wq_sb[:, 2 * j : 2 * j + 2, hp * 128 : (hp + 1) * 128],
                            qtl_sb[:, 2 * j : 2 * j + 2, :],
                            start=(j == 0), stop=(j == NKC // 2 - 1),
                            perf_mode=DR,
                        )
                    evac(q_sb[:, hp, :], ps[:], 1.0 / (SW * 8.0),
                         bq_sb[:, hp : hp + 1] if use_bias else None)

                qproj(0)
                kproj(0)
                vproj(0)
                vproj(1)
                vproj(2)
                for hp in range(HPAIR):
                    cps = cpsp.tile([128, 2, 512], f32, name="cps", tag="cps")
                    pend = []
                    for kc in range(NTC):
                        if hp == 0 and 2 < kc + 3 < NTC:
                            vproj(kc + 3)
                        if kc == 2 and hp < HPAIR - 1:
                            kproj(hp + 1)
                        if kc == 6 and hp < HPAIR - 1:
                            qproj(hp + 1)
                        sp = spsp.tile([128, 2, 512], f32, name="sp", tag="sp")
                        for h2 in range(2):
                            nc.tensor.matmul(
                                sp[:, h2, :],
                                k_sb[64 * h2 : 64 * (h2 + 1), hp, kc * 128 : (kc + 1) * 128],
                                q_sb[64 * h2 : 64 * (h2 + 1), hp, :],
                                start=True, stop=True,
                            )
                        e = ep.tile([128, 2, 512], bf16, name="e", tag="e")
                        if kc in SCHRAUD_KCS:
                            # Schraudolph exp on DVE: bf16 bits of e^x are
                            # ~ x*128/ln2 + 127*128 (softmax cancels the bias)
                            nc.vector.tensor_scalar(
                                out=e[:].bitcast(i16), in0=sp[:],
                                scalar1=184.6650, scalar2=16256.0,
                                op0=MUL, op1=ADD)
                        else:
                            nc.scalar.activation(e[:], sp[:], Exp)
                        a = ap_.tile([128, 2, 512], bf16, name="a", tag="a")
                        mb = mask_sb[:, kc : kc + 1, :]
                        dcol = 512 - POOL_MASK_COLS
                        nc.vector.tensor_mul(
                            a[:, :, 0:dcol], e[:, :, 0:dcol],
                            mb[:, :, 0:dcol].broadcast_to((128, 2, dcol)))
                        nc.gpsimd.tensor_mul(
                            a[:, :, dcol:512], e[:, :, dcol:512],
                            mb[:, :, dcol:512].broadcast_to(
                                (128, 2, POOL_MASK_COLS)))
                        # context matmuls pipelined one kc behind the scores
                        # so the scalar engine can run ahead
                        pend.append((a, kc))
                        if len(pend) > 1 or kc == NTC - 1:
                            todo = pend if kc == NTC - 1 else pend[:1]
                            for pa, pkc in todo:
                                for h2 in range(2):
                                    h = 2 * hp + h2
                                    for qc in range(4):
                                        nc.tensor.matmul(
                                            cps[:, h2, 65 * qc : 65 * qc + 65],
                                            pa[:, h2, qc * 128 : (qc + 1) * 128],
                                            v_sb[:, pkc, h, :],
                                            start=(pkc == 0), stop=(pkc == NTC - 1),
                                        )
                            del pend[: len(todo)]
                    # normalize + evacuate context (natural layout)
                    rec = nrm.tile([128, 2, 4, 1], f32, name="rec", tag="rec")
                    nc.vector.reciprocal(
                        rec[:],
                        cps[:, :, 0 : 4 * VW].rearrange(
                            "p h (q w) -> p h q w", w=VW)[:, :, :, DK : DK + 1])
                    for qc in range(4):
                        for h2 in range(2):
                            h = 2 * hp + h2
                            if (CTXNORM_ACT >> h2) & 1:
                                nc.scalar.activation(
                                    ctxn[:, qc, h * DK : (h + 1) * DK],
                                    cps[:, h2, 65 * qc : 65 * qc + DK],
                                    Copy, scale=rec[:, h2, qc, :])
                            else:
                                nc.vector.tensor_scalar(
                                    out=ctxn[:, qc, h * DK : (h + 1) * DK],
                                    in0=cps[:, h2, 65 * qc : 65 * qc + DK],
                                    scalar1=rec[:, h2, qc, :],
                                    scalar2=None, op0=MUL)
                        if hp < HPAIR - 1:
                            # transpose this (head pair, q-chunk): DMA xbar
                            nc.sync.dma_start_transpose(
                                ctxt[:, hp, qc * 128 : (qc + 1) * 128],
                                ctxn[:, qc, hp * 128 : (hp + 1) * 128])
                        else:
                            # tail: PE transpose + ScalarE fp8 evac is lower
                            # latency than the DMA xbar path
                            if qc == 0:
                                tps = cpsp.tile([128, 4, 512], bf16,
                                                name="tps", tag="cps")
                            nc.tensor.transpose(
                                tps[:, qc, 0:128],
                                ctxn[:, qc, hp * 128 : (hp + 1) * 128],
                                ident_sb[:])
                            nc.scalar.activation(
                                ctxt8[:, hp, qc * 128 : (qc + 1) * 128],
                                tps[:, qc, 0:128], Copy)
                    if hp < HPAIR - 1:
                        nc.vector.tensor_copy(out=ctxt8[:, hp, :], in_=ctxt[:, hp, :])

            # ================= Phase 3: O-proj + residual + LN ============
            with (
                tc.tile_pool(name="ops", bufs=3, space="PSUM") as opsp,
                tc.tile_pool(name="ost", bufs=1, space="PSUM") as ostp,
                tc.tile_pool(name="oln", bufs=4) as oln,
            ):
                ost = ostp.tile([128, 4], f32, name="ost", tag="ost")
                for tcn in range(TOK // 128):
                    pso = opsp.tile([128, 2, 512], f32, name="pso", tag="pso")
                    for dc in range(2):
                        for j in range(NKC // 2):
                            nc.tensor.matmul(
                                pso[:, dc, :],
                                ctxt8[:, 2 * j : 2 * j + 2, tcn * 128 : (tcn + 1) * 128],
                                wo_sb[:, 2 * j : 2 * j + 2, dc * 512 : (dc + 1) * 512],
                                start=(j == 0), stop=(j == NKC // 2 - 1),
                                perf_mode=DR,
                            )
                    # row-sums of the o-projection (for the LN mean), via the
                    # same DR contraction against host-provided wo row-sums
                    for j in range(NKC // 2):
                        nc.tensor.matmul(
                            ost[:, tcn : tcn + 1],
                            ctxt8[:, 2 * j : 2 * j + 2, tcn * 128 : (tcn + 1) * 128],
                            wors_sb[:, 2 * j : 2 * j + 2, :],
                            start=(j == 0), stop=(j == NKC // 2 - 1),
                            perf_mode=DR,
                        )
                    osb = oln.tile([128, D], f32, name="osb", tag="osb")
                    nc.vector.tensor_add(
                        out=osb.rearrange("p (a b) -> p a b", a=2),
                        in0=pso[:],
                        in1=qres_sb[:, tcn, :].rearrange("p (a b) -> p a b", a=2))
                    mean = oln.tile([128, 1], f32, name="mean", tag="mean")
                    nc.vector.tensor_scalar(
                        out=mean[:], in0=ost[:, tcn : tcn + 1],
                        scalar1=1.0 / D, scalar2=qrsm_sb[:, tcn : tcn + 1],
                        op0=MUL, op1=ADD)
                    y = oln.tile([128, D], f32, name="y", tag="y")
                    sumsq = oln.tile([128, 1], f32, name="sumsq", tag="sumsq")
                    nc.scalar.activation(y[:], osb[:], Square, accum_out=sumsq[:])
                    var = oln.tile([128, 1], f32, name="var", tag="var")
                    nc.vector.tensor_scalar(
                        out=var[:], in0=mean[:], scalar1=mean[:],
                        scalar2=eps_t[:], op0=MUL, op1=SUB)
                    # var now holds mu^2 - eps; rstd = 1/sqrt(sumsq/D - var)
                    nc.vector.tensor_scalar(
                        out=var[:], in0=sumsq[:], scalar1=1.0 / D,
                        scalar2=var[:], op0=MUL, op1=SUB)
                    rstd = oln.tile([128, 1], f32, name="rstd", tag="rstd")
                    nc.scalar.activation(rstd[:], var[:], Sqrt)
                    nc.vector.reciprocal(rstd[:], rstd[:])
                    for half in range(2):
                        hs_ = slice(half * 512, (half + 1) * 512)
                        nc.vector.tensor_scalar(
                            out=y[:, hs_], in0=osb[:, hs_],
                            scalar1=mean[:], scalar2=rstd[:],
                            op0=SUB, op1=MUL)
                        if use_gb:
                            nc.vector.tensor_mul(y[:, hs_], y[:, hs_], gamma_bc[:, hs_])
                            nc.vector.tensor_add(y[:, hs_], y[:, hs_], beta_bc[:, hs_])
                        nc.sync.dma_start(
                            out[tcn * 128 : (tcn + 1) * 128, hs_], y[:, hs_])

    nc.compile()
    return nc


def _get_nc(use_bias=False, use_gb=False):
    key = ("nc", use_bias, use_gb)
    if key not in _CACHE:
        _CACHE[key] = _build_nc(use_bias=use_bias, use_gb=use_gb)
    return _CACHE[key]


def _to_f8(a):
    return np.clip(np.asarray(a, np.float32), -240.0, 240.0).astype(
        ml_dtypes.float8_e4m3)


def make_in_maps(inputs):
    Q = np.asarray(inputs["Q"], np.float32)
    mask = np.asarray(inputs["attn_mask"])
    Wq = np.asarray(inputs["Wq"], np.float32)
    Wk = np.asarray(inputs["Wk"], np.float32)
    Wv = np.asarray(inputs["Wv"], np.float32)
    Wo = np.asarray(inputs["Wo"], np.float32)
    bq = np.asarray(inputs["bq"], np.float32)
    bk = np.asarray(inputs["bk"], np.float32)
    bv = np.asarray(inputs["bv"], np.float32)
    bo = np.asarray(inputs["bo"], np.float32)
    gamma = np.asarray(inputs["gamma"], np.float32)
    beta = np.asarray(inputs["beta"], np.float32)

    use_bias = any(np.any(x != 0) for x in (bq, bk, bv))
    use_gb = bool(np.any(gamma != 1) or np.any(beta != 0))

    wq8 = _to_f8(Wq * SW)
    wk8 = _to_f8(Wk * SW)
    wv8 = _to_f8(Wv * SW)
    wo8 = _to_f8(Wo * SW)
    # row-sums of the *quantized* wo, for the LN mean shortcut
    wors8 = _to_f8(wo8.astype(np.float32).sum(axis=1))[:, None]

    in_maps = []
    for c in range(NCORES):
        b, g = c // G, c % G
        qtb = _to_f8(Q[b].T)
        qtlb = np.ascontiguousarray(qtb[:, g * TOK : (g + 1) * TOK])
        mtb = np.ascontiguousarray(
            (~mask[b])[:, g * TOK : (g + 1) * TOK]).astype(ml_dtypes.bfloat16)
        m = {
            "qt": qtb,
            "qtl": qtlb,
            "maskt": mtb,
            "wq": wq8,
            "wk": wk8,
            "wv": wv8,
            "wo": wo8,
            "qres": (np.ascontiguousarray(Q[b, g * TOK : (g + 1) * TOK]) + bo) * SW,
            "wors": wors8,
        }
        m["qrsm"] = m["qres"].sum(axis=1) / float(D)
        m["ident"] = np.eye(128, dtype=np.float32).astype(ml_dtypes.bfloat16)
        if use_bias:
            m["bq"] = bq / 8.0
            m["bk"] = bk
            m["bv"] = bv[None, :].astype(ml_dtypes.bfloat16)
        if use_gb:
            m["gamma"] = gamma
            m["beta"] = beta
        in_maps.append(m)
    return in_maps, use_bias, use_gb


def kernel(**inputs):
    from concourse.bass_utils import run_bass_kernel_spmd

    in_maps, use_bias, use_gb = make_in_maps(inputs)
    nc = _get_nc(use_bias, use_gb)
    res = run_bass_kernel_spmd(nc, in_maps, core_ids=list(range(NCORES)))
    out = np.empty((B, S, D), np.float32)
    for c in range(NCORES):
        b, g = c // G, c % G
        out[b, g * TOK : (g + 1) * TOK] = res.results[c]["out"]
    return out
